# revision 2
# baseline (speedup 1.0000x reference)
"""Chamfer distance kernel for Trainium2 (8 NeuronCores, batch-parallel).

Per core (one batch): -D2 = 2*x1.x2 - n1[n] - n2[m] computed on the PE as a
single K=13 float32r matmul whose contraction rows carry the hi/lo limb split
of the coordinates and of both squared norms (hi+lo reconstructs fp32
exactly, so D2 is fp32-accurate up to the dropped lo*lo term ~2^-26). Signs
are flipped so both reductions are MAX.

Structure (v2, trace-driven):
- Setup: per side, all 11 data rows are pre-assembled in natural layout into
  one XS tile ([128, 11*32] fp32, every value already carrying its final
  f32r-rounded bit pattern), then scattered into the [13, 4096] operand rows
  by 11 single-row DMAs + 1 const-row DMA, spread over the 3 DMA queues
  (SP/ACT HWDGE + gpsimd SWDGE). The Sqrt activation table is preloaded.
- Main loop (HW-measured DVE-instruction-issue-bound at ~240ns/inst): both
  2048-wide PSUM groups evac (ScalarE fp32->bf16, 2x elem rate on HW) into
  ONE [128, 4096] bf16 C tile, so each I-tile reduces with 5 DVE
  instructions: colmax accumulate (4096-wide bf16 tensor_tensor, ~4x mode),
  rowmax pair-tree 2048/1024/512, and a 512-wide tensor_reduce into
  rmall[:, it]. Runs at ~1.72us/I-tile = the PE streaming roofline.
- Tail: rowmin path (sqrt with the clamp fused into the activation bias)
  issues while gpsimd partition-reduces the colmax accumulator in two
  2048-wide halves; each half's natural-layout gather DMA goes on its own
  HWDGE queue to overlap trigger latency.

loop_n > 1 replicates for differential HW timing: FULL=1 wraps the whole
body (staging+main+tail), FULL=0 only the main loop.
"""

import sys

sys.path.insert(0, "/opt/trn_rl_repo")

import numpy as np
from contextlib import ExitStack, nullcontext

import concourse.bacc as bacc
import concourse.tile as tile
import concourse.bass_isa as bass_isa
from concourse import mybir
from concourse.bass_utils import run_bass_kernel_spmd

B, NPTS, KDIM = 8, 4096, 3
IT_N = NPTS // 128   # 32 I-tiles of 128 rows (x1 points)
GSPAN = 2048         # PSUM group span (4 banks)
NG = NPTS // GSPAN   # 2 groups per I-tile

F32 = mybir.dt.float32
F32R = mybir.dt.float32r
BF16 = mybir.dt.bfloat16

_cached = {}


def _build(reps: int = 1, loop_n: int = 1, TSTOP: int = 256, FULL: int = 0):
    nc = bacc.Bacc("TRN2", target_bir_lowering=False, debug=False, num_devices=B)

    x1_d = nc.dram_tensor("x1", [NPTS, KDIM], F32, kind="ExternalInput").ap()
    x2_d = nc.dram_tensor("x2", [NPTS, KDIM], F32, kind="ExternalInput").ap()
    outc_d = nc.dram_tensor("outc", [128, IT_N], F32, kind="ExternalOutput").ap()
    outr_d = nc.dram_tensor("outr", [128, IT_N], F32, kind="ExternalOutput").ap()

    MX = mybir.AluOpType.max
    AD = mybir.AluOpType.add
    X = mybir.AxisListType.X
    KROWS = 13
    SQRT = mybir.ActivationFunctionType.Sqrt

    with tile.TileContext(nc) as tc, ExitStack() as ctx:
        sb = ctx.enter_context(tc.tile_pool(name="sb", bufs=1))
        scr = ctx.enter_context(tc.tile_pool(name="scr", bufs=6))
        cbp = ctx.enter_context(tc.tile_pool(name="cbp", bufs=3))
        trp = ctx.enter_context(tc.tile_pool(name="trp", bufs=2))
        ps = ctx.enter_context(tc.tile_pool(name="ps", bufs=2, space="PSUM"))
        xsp = ctx.enter_context(tc.tile_pool(name="xsp", bufs=2))
        tlp = ctx.enter_context(tc.tile_pool(name="tlp", bufs=2))

        # sqrt table preload (ACT idle during setup; avoids the 1.3us
        # LoadActFuncSet stall in the tail); eps doubles as the fused-clamp
        # bias for the tail sqrt
        eps = sb.tile([128, 1], F32)
        nc.vector.memset(eps[:], 1e-6)
        zo = scr.tile([128, 1], F32, tag="zo")
        nc.scalar.activation(zo[:], eps[:], SQRT, scale=1.0)

        qs = [nc.sync, nc.gpsimd, nc.scalar]
        ones_c = scr.tile([128, 64], F32, tag="cns")
        nc.gpsimd.memset(ones_c[:], 1.0)
        mones_c = scr.tile([128, 64], F32, tag="cns")
        nc.gpsimd.memset(mones_c[:], -1.0)

        L = sb.tile([KROWS, NPTS], F32R)
        R = sb.tile([KROWS, NPTS], F32R)
        cmA = sb.tile([128, NPTS], BF16)
        cmB = sb.tile([128, NPTS], BF16)
        rmall = sb.tile([128, IT_N], F32)

        # Row k of L/R receives one 32-wide XS block; point n = 32*p + t.
        # L rows: 0-2 x1hi, 3-5 x1hi, 6-8 x1lo, 9 n1hi, 10 n1lo, 11-12 +1
        # R rows: 0-2 (2x2)hi, 3-5 (2x2)lo, 6-8 (2x2)hi, 9-10 -1,
        #         11 (-n2)hi, 12 (-n2)lo
        def stage_side(x_d, S_dst, scale, norm_factor, hi_blocks, lo_blocks,
                       n_blocks, const_blocks, const_tile, qoff):
            xn = scr.tile([128, 96], F32, tag="nat")
            nc.sync.dma_start(xn[:], x_d.rearrange("(p t) k -> p (t k)", p=128))
            if scale != 1.0:
                nc.vector.tensor_scalar_mul(xn[:], xn[:], scale)
            hi96 = scr.tile([128, 96], F32R, tag="nat")
            nc.vector.tensor_copy(hi96[:], xn[:])
            lo96 = scr.tile([128, 96], F32, tag="nat")
            nc.vector.tensor_sub(lo96[:], xn[:], hi96[:].bitcast(F32))
            sq96 = scr.tile([128, 96], F32, tag="nat")
            nc.vector.tensor_mul(sq96[:], xn[:], xn[:])
            nn = scr.tile([128, 32], F32, tag="natn")
            nc.vector.tensor_reduce(
                nn[:], sq96[:].rearrange("p (t k) -> p t k", k=KDIM), axis=X, op=AD
            )
            f = norm_factor / (scale * scale)
            if f != 1.0:
                nc.vector.tensor_scalar_mul(nn[:], nn[:], f)
            nhi = scr.tile([128, 32], F32R, tag="natn")
            nc.vector.tensor_copy(nhi[:], nn[:])
            nlo = scr.tile([128, 32], F32, tag="natn")
            nc.vector.tensor_sub(nlo[:], nn[:], nhi[:].bitcast(F32))

            # one contiguous 32-wide XS block per destination row keeps each
            # scatter DMA at 128 descriptors
            XS = xsp.tile([128, 11 * 32], F32, tag="xs")
            hiv = hi96[:].bitcast(F32).rearrange("p (t k) -> p t k", k=KDIM)
            lov = lo96[:].rearrange("p (t k) -> p t k", k=KDIM)
            rows = []  # dst row of XS block i
            b = 0
            for base in hi_blocks:
                for k in range(KDIM):
                    nc.vector.tensor_copy(XS[:, b * 32:(b + 1) * 32], hiv[:, :, k])
                    rows.append(base + k)
                    b += 1
            for k in range(KDIM):
                nc.vector.tensor_copy(XS[:, b * 32:(b + 1) * 32], lov[:, :, k])
                rows.append(lo_blocks + k)
                b += 1
            nc.vector.tensor_copy(XS[:, b * 32:(b + 1) * 32], nhi[:].bitcast(F32))
            rows.append(n_blocks)
            b += 1
            nc.vector.tensor_copy(XS[:, b * 32:(b + 1) * 32], nlo[:])
            rows.append(n_blocks + 1)
            for i, r in enumerate(rows):
                qs[(qoff + i) % len(qs)].dma_start(
                    S_dst[r:r + 1, :].bitcast(F32).rearrange(
                        "o (p t) -> o p t", p=128),
                    XS[:, i * 32:(i + 1) * 32],
                )
            c0, c1 = const_blocks
            qs[(qoff + len(rows)) % len(qs)].dma_start(
                S_dst[c0:c1, :].bitcast(F32),
                const_tile[:, 0:(c1 - c0) * 32],
            )

        def emit_stage():
            stage_side(x1_d, L, 1.0, 1.0, (0, 3), 6, 9, (11, 13), ones_c, 0)
            stage_side(x2_d, R, 2.0, -1.0, (0, 6), 3, 11, (9, 11), mones_c, 2)

        def emit_main():
            nc.gpsimd.memset(cmA[:], -3.0e38)
            for it in range(IT_N):
                # both PSUM groups evac into one [128, 4096] C tile so the
                # whole I-tile reduces with 5 DVE instructions (the HW main
                # loop is DVE-instruction-issue-bound at ~240ns/inst)
                C = cbp.tile([128, NPTS], BF16)
                for g in range(NG):
                    P = ps.tile([128, GSPAN], F32)
                    for j in range(GSPAN // 512):
                        nc.tensor.matmul(
                            P[:, j * 512:(j + 1) * 512],
                            L[:, it * 128:(it + 1) * 128],
                            R[:, (g * (GSPAN // 512) + j) * 512:
                                 (g * (GSPAN // 512) + j + 1) * 512],
                            start=True,
                            stop=True,
                        )
                    nc.scalar.copy(C[:, g * GSPAN:(g + 1) * GSPAN], P[:])
                src, dst = (cmA, cmB) if it % 2 == 0 else (cmB, cmA)
                nc.vector.tensor_tensor(dst[:], src[:], C[:], op=MX)
                w = NPTS // 2
                prev = C
                while w > TSTOP:
                    t2 = trp.tile([128, w], BF16, tag=f"tr{w}")
                    nc.vector.tensor_tensor(
                        t2[:], prev[:, 0:w], prev[:, w:2 * w], op=MX
                    )
                    prev = t2
                    w //= 2
                nc.vector.tensor_reduce(
                    rmall[:, it:it + 1], prev[:, 0:2 * TSTOP], axis=X, op=MX
                )

        def emit_tail():
            # rowmin path first: overlaps the gpsimd partition reduces
            nc.vector.tensor_scalar_min(rmall[:], rmall[:], 0.0)
            o1 = tlp.tile([128, IT_N], F32, tag="o1")
            nc.scalar.activation(o1[:], rmall[:], SQRT, scale=-1.0)
            nc.scalar.dma_start(outr_d[:], o1[:])

            cm_fin = cmA  # IT_N even: final colmax lands back in cmA
            cmd = tlp.tile([128, IT_N], BF16, tag="cmd")
            PPH = GSPAN // 32  # partitions of cmd covered per half (64)
            gq = [nc.sync, nc.scalar]
            for h in range(NG):
                cmr = tlp.tile([128, GSPAN], BF16, tag=f"cmr{h}")
                nc.gpsimd.partition_all_reduce(
                    cmr[:], cm_fin[:, h * GSPAN:(h + 1) * GSPAN], channels=128,
                    reduce_op=bass_isa.ReduceOp.max
                )
                gq[h % 2].dma_start(
                    cmd[h * PPH:(h + 1) * PPH, :],
                    cmr[0:1, :].rearrange("o (p t) -> o p t", p=PPH),
                )
            nc.vector.tensor_scalar_min(cmd[:], cmd[:], 0.0)
            o0 = tlp.tile([128, IT_N], F32, tag="o0")
            nc.scalar.activation(o0[:], cmd[:], SQRT, scale=-1.0)
            nc.sync.dma_start(outc_d[:], o0[:])

        # FULL placement of phases relative to the timing For_i:
        #   0: stage+tail outside, loop=main (legacy main-only slope)
        #   1: loop = stage+main+tail (true per-exec slope)
        #   2: stage outside, loop = main+tail
        #   3: tail outside, loop = stage+main
        if loop_n > 1:
            if FULL == 1:
                with tc.For_i(0, loop_n, 1):
                    emit_stage()
                    emit_main()
                    emit_tail()
            elif FULL == 2:
                emit_stage()
                with tc.For_i(0, loop_n, 1):
                    emit_main()
                    emit_tail()
            elif FULL == 3:
                with tc.For_i(0, loop_n, 1):
                    emit_stage()
                    emit_main()
                emit_tail()
            else:
                emit_stage()
                with tc.For_i(0, loop_n, 1):
                    emit_main()
                emit_tail()
        else:
            emit_stage()
            emit_main()
            emit_tail()

    nc.compile()
    return nc


def _get(reps: int = 1, loop_n: int = 1, **kw):
    key = (reps, loop_n, tuple(sorted(kw.items())))
    if key not in _cached:
        _cached[key] = _build(reps, loop_n, **kw)
    return _cached[key]


def kernel(input1: np.ndarray, input2: np.ndarray, _trace: bool = False, **kw):
    nc = _get(**kw)
    input1 = np.ascontiguousarray(np.asarray(input1, dtype=np.float32))
    input2 = np.ascontiguousarray(np.asarray(input2, dtype=np.float32))
    in_maps = [{"x1": input1[b], "x2": input2[b]} for b in range(B)]
    res = run_bass_kernel_spmd(nc, in_maps, core_ids=list(range(B)), trace=_trace)
    losses = []
    for b in range(B):
        r = res.results[b]
        losses.append(
            r["outc"].mean(dtype=np.float64) + r["outr"].mean(dtype=np.float64)
        )
    out = np.float32(np.mean(losses))
    if _trace:
        return out, res
    return out


# revision 3
# speedup vs baseline: 1.0445x; 1.0445x over previous
"""Chamfer distance kernel for Trainium2 (8 NeuronCores, batch-parallel).

Per core (one batch): -D2 = 2*x1.x2 - n1[n] - n2[m] computed on the PE as a
single K=13 float32r matmul whose contraction rows carry the hi/lo limb split
of the coordinates and of both squared norms (hi+lo reconstructs fp32
exactly, so D2 is fp32-accurate up to the dropped lo*lo term ~2^-26). Signs
are flipped so both reductions are MAX.

Structure (v2, trace-driven):
- Setup: per side, all 11 data rows are pre-assembled in natural layout into
  one XS tile ([128, 11*32] fp32, every value already carrying its final
  f32r-rounded bit pattern), then scattered into the [13, 4096] operand rows
  by 11 single-row DMAs + 1 const-row DMA, spread over the 3 DMA queues
  (SP/ACT HWDGE + gpsimd SWDGE). The Sqrt activation table is preloaded.
- Main loop (HW-measured DVE-instruction-issue-bound at ~240ns/inst): both
  2048-wide PSUM groups evac (ScalarE fp32->bf16, 2x elem rate on HW) into
  ONE [128, 4096] bf16 C tile, so each I-tile reduces with 5 DVE
  instructions: colmax accumulate (4096-wide bf16 tensor_tensor, ~4x mode),
  rowmax pair-tree 2048/1024/512, and a 512-wide tensor_reduce into
  rmall[:, it]. Runs at ~1.72us/I-tile = the PE streaming roofline.
- Tail: rowmin path (sqrt with the clamp fused into the activation bias)
  issues while gpsimd partition-reduces the colmax accumulator in two
  2048-wide halves; each half's natural-layout gather DMA goes on its own
  HWDGE queue to overlap trigger latency.

loop_n > 1 replicates for differential HW timing: FULL=1 wraps the whole
body (staging+main+tail), FULL=0 only the main loop.
"""

import sys

sys.path.insert(0, "/opt/trn_rl_repo")

import numpy as np
from contextlib import ExitStack, nullcontext

import concourse.bacc as bacc
import concourse.tile as tile
import concourse.bass_isa as bass_isa
from concourse import mybir
from concourse.bass_utils import run_bass_kernel_spmd

B, NPTS, KDIM = 8, 4096, 3
IT_N = NPTS // 128   # 32 I-tiles of 128 rows (x1 points)
GSPAN = 2048         # PSUM group span (4 banks)
NG = NPTS // GSPAN   # 2 groups per I-tile

F32 = mybir.dt.float32
F32R = mybir.dt.float32r
BF16 = mybir.dt.bfloat16

_cached = {}


def _build(reps: int = 1, loop_n: int = 1, TSTOP: int = 256, FULL: int = 0):
    nc = bacc.Bacc("TRN2", target_bir_lowering=False, debug=False, num_devices=B)

    x1_d = nc.dram_tensor("x1", [NPTS, KDIM], F32, kind="ExternalInput").ap()
    x2_d = nc.dram_tensor("x2", [NPTS, KDIM], F32, kind="ExternalInput").ap()
    outc_d = nc.dram_tensor("outc", [128, IT_N], F32, kind="ExternalOutput").ap()
    outr_d = nc.dram_tensor("outr", [128, IT_N], F32, kind="ExternalOutput").ap()

    MX = mybir.AluOpType.max
    AD = mybir.AluOpType.add
    X = mybir.AxisListType.X
    KROWS = 13
    SQRT = mybir.ActivationFunctionType.Sqrt

    with tile.TileContext(nc) as tc, ExitStack() as ctx:
        sb = ctx.enter_context(tc.tile_pool(name="sb", bufs=1))
        scr = ctx.enter_context(tc.tile_pool(name="scr", bufs=6))
        cbp = ctx.enter_context(tc.tile_pool(name="cbp", bufs=3))
        trp = ctx.enter_context(tc.tile_pool(name="trp", bufs=2))
        ps = ctx.enter_context(tc.tile_pool(name="ps", bufs=2, space="PSUM"))
        xsp = ctx.enter_context(tc.tile_pool(name="xsp", bufs=2))
        tlp = ctx.enter_context(tc.tile_pool(name="tlp", bufs=2))

        # sqrt table preload (ACT idle during setup; avoids the 1.3us
        # LoadActFuncSet stall in the tail); eps doubles as the fused-clamp
        # bias for the tail sqrt
        eps = sb.tile([128, 1], F32)
        nc.vector.memset(eps[:], 1e-6)
        zo = scr.tile([128, 1], F32, tag="zo")
        nc.scalar.activation(zo[:], eps[:], SQRT, scale=1.0)

        qs = [nc.sync, nc.gpsimd, nc.scalar]
        ones_c = scr.tile([128, 64], F32, tag="cns")
        nc.gpsimd.memset(ones_c[:], 1.0)
        mones_c = scr.tile([128, 64], F32, tag="cns")
        nc.gpsimd.memset(mones_c[:], -1.0)

        L = sb.tile([KROWS, NPTS], F32R)
        R = sb.tile([KROWS, NPTS], F32R)
        cmA = sb.tile([128, NPTS], BF16)
        cmB = sb.tile([128, NPTS], BF16)
        rmall = sb.tile([128, IT_N], F32)

        # Row k of L/R receives one 32-wide XS block; point n = 32*p + t.
        # L rows: 0-2 x1hi, 3-5 x1hi, 6-8 x1lo, 9 n1hi, 10 n1lo, 11-12 +1
        # R rows: 0-2 (2x2)hi, 3-5 (2x2)lo, 6-8 (2x2)hi, 9-10 -1,
        #         11 (-n2)hi, 12 (-n2)lo
        def stage_side(x_d, S_dst, scale, norm_factor, hi_blocks, lo_blocks,
                       n_blocks, const_blocks, const_tile, qoff):
            # 32-partition natural layout (partition = n//128): per-point math
            # still uses all lanes' worth of free-dim throughput, and each
            # scatter DMA needs only 32 descriptors of 512B (vs 128x128B when
            # staging on 128 partitions). Column order n is unchanged.
            NW = NPTS // 128  # 32 partitions, 128 points each
            xn = scr.tile([NW, 384], F32, tag="nat")
            nc.sync.dma_start(xn[:], x_d.rearrange("(t p) k -> t (p k)", t=NW))
            if scale != 1.0:
                nc.vector.tensor_scalar_mul(xn[:], xn[:], scale)
            hi3 = scr.tile([NW, 384], F32R, tag="nat")
            nc.vector.tensor_copy(hi3[:], xn[:])
            lo3 = scr.tile([NW, 384], F32, tag="nat")
            nc.vector.tensor_sub(lo3[:], xn[:], hi3[:].bitcast(F32))
            sq3 = scr.tile([NW, 384], F32, tag="nat")
            nc.vector.tensor_mul(sq3[:], xn[:], xn[:])
            nn = scr.tile([NW, 128], F32, tag="natn")
            nc.vector.tensor_reduce(
                nn[:], sq3[:].rearrange("p (t k) -> p t k", k=KDIM), axis=X, op=AD
            )
            f = norm_factor / (scale * scale)
            if f != 1.0:
                nc.vector.tensor_scalar_mul(nn[:], nn[:], f)
            nhi = scr.tile([NW, 128], F32R, tag="natn")
            nc.vector.tensor_copy(nhi[:], nn[:])
            nlo = scr.tile([NW, 128], F32, tag="natn")
            nc.vector.tensor_sub(nlo[:], nn[:], nhi[:].bitcast(F32))

            XS = xsp.tile([NW, 11 * 128], F32, tag="xs")
            hiv = hi3[:].bitcast(F32).rearrange("p (t k) -> p t k", k=KDIM)
            lov = lo3[:].rearrange("p (t k) -> p t k", k=KDIM)
            rows = []  # dst row of XS block i
            b = 0
            for base in hi_blocks:
                for k in range(KDIM):
                    nc.vector.tensor_copy(XS[:, b * 128:(b + 1) * 128],
                                          hiv[:, :, k])
                    rows.append(base + k)
                    b += 1
            for k in range(KDIM):
                nc.vector.tensor_copy(XS[:, b * 128:(b + 1) * 128], lov[:, :, k])
                rows.append(lo_blocks + k)
                b += 1
            nc.vector.tensor_copy(XS[:, b * 128:(b + 1) * 128], nhi[:].bitcast(F32))
            rows.append(n_blocks)
            b += 1
            nc.vector.tensor_copy(XS[:, b * 128:(b + 1) * 128], nlo[:])
            rows.append(n_blocks + 1)
            for i, r in enumerate(rows):
                qs[(qoff + i) % len(qs)].dma_start(
                    S_dst[r:r + 1, :].bitcast(F32).rearrange(
                        "o (t p) -> o t p", t=NW),
                    XS[:, i * 128:(i + 1) * 128],
                )
            c0, c1 = const_blocks
            qs[(qoff + len(rows)) % len(qs)].dma_start(
                S_dst[c0:c1, :].bitcast(F32),
                const_tile[:, 0:(c1 - c0) * 32],
            )

        def emit_stage():
            stage_side(x1_d, L, 1.0, 1.0, (0, 3), 6, 9, (11, 13), ones_c, 0)
            stage_side(x2_d, R, 2.0, -1.0, (0, 6), 3, 11, (9, 11), mones_c, 2)

        def emit_main():
            nc.gpsimd.memset(cmA[:], -3.0e38)
            for it in range(IT_N):
                # both PSUM groups evac into one [128, 4096] C tile so the
                # whole I-tile reduces with 5 DVE instructions (the HW main
                # loop is DVE-instruction-issue-bound at ~240ns/inst)
                C = cbp.tile([128, NPTS], BF16)
                for g in range(NG):
                    P = ps.tile([128, GSPAN], F32)
                    for j in range(GSPAN // 512):
                        nc.tensor.matmul(
                            P[:, j * 512:(j + 1) * 512],
                            L[:, it * 128:(it + 1) * 128],
                            R[:, (g * (GSPAN // 512) + j) * 512:
                                 (g * (GSPAN // 512) + j + 1) * 512],
                            start=True,
                            stop=True,
                        )
                    nc.scalar.copy(C[:, g * GSPAN:(g + 1) * GSPAN], P[:])
                src, dst = (cmA, cmB) if it % 2 == 0 else (cmB, cmA)
                if it == IT_N - 1:
                    # split the last colmax so the tail's partition reduce of
                    # half 0 starts while half 1 still accumulates
                    nc.vector.tensor_tensor(
                        dst[:, 0:GSPAN], src[:, 0:GSPAN], C[:, 0:GSPAN], op=MX
                    )
                    nc.vector.tensor_tensor(
                        dst[:, GSPAN:], src[:, GSPAN:], C[:, GSPAN:], op=MX
                    )
                else:
                    nc.vector.tensor_tensor(dst[:], src[:], C[:], op=MX)
                w = NPTS // 2
                prev = C
                while w > TSTOP:
                    t2 = trp.tile([128, w], BF16, tag=f"tr{w}")
                    nc.vector.tensor_tensor(
                        t2[:], prev[:, 0:w], prev[:, w:2 * w], op=MX
                    )
                    prev = t2
                    w //= 2
                nc.vector.tensor_reduce(
                    rmall[:, it:it + 1], prev[:, 0:2 * TSTOP], axis=X, op=MX
                )

        def emit_tail():
            # rowmin path first: overlaps the gpsimd partition reduces
            nc.vector.tensor_scalar_min(rmall[:], rmall[:], 0.0)
            o1 = tlp.tile([128, IT_N], F32, tag="o1")
            nc.scalar.activation(o1[:], rmall[:], SQRT, scale=-1.0)
            nc.scalar.dma_start(outr_d[:], o1[:])

            cm_fin = cmA  # IT_N even: final colmax lands back in cmA
            cmd = tlp.tile([128, IT_N], BF16, tag="cmd")
            PPH = GSPAN // 32  # partitions of cmd covered per half (64)
            gq = [nc.sync, nc.scalar]
            for h in range(NG):
                cmr = tlp.tile([128, GSPAN], BF16, tag=f"cmr{h}")
                nc.gpsimd.partition_all_reduce(
                    cmr[:], cm_fin[:, h * GSPAN:(h + 1) * GSPAN], channels=128,
                    reduce_op=bass_isa.ReduceOp.max
                )
                gq[h % 2].dma_start(
                    cmd[h * PPH:(h + 1) * PPH, :],
                    cmr[0:1, :].rearrange("o (p t) -> o p t", p=PPH),
                )
            nc.vector.tensor_scalar_min(cmd[:], cmd[:], 0.0)
            o0 = tlp.tile([128, IT_N], F32, tag="o0")
            nc.scalar.activation(o0[:], cmd[:], SQRT, scale=-1.0)
            nc.sync.dma_start(outc_d[:], o0[:])

        # FULL placement of phases relative to the timing For_i:
        #   0: stage+tail outside, loop=main (legacy main-only slope)
        #   1: loop = stage+main+tail (true per-exec slope)
        #   2: stage outside, loop = main+tail
        #   3: tail outside, loop = stage+main
        if loop_n > 1:
            if FULL == 1:
                with tc.For_i(0, loop_n, 1):
                    emit_stage()
                    emit_main()
                    emit_tail()
            elif FULL == 2:
                emit_stage()
                with tc.For_i(0, loop_n, 1):
                    emit_main()
                    emit_tail()
            elif FULL == 3:
                with tc.For_i(0, loop_n, 1):
                    emit_stage()
                    emit_main()
                emit_tail()
            else:
                emit_stage()
                with tc.For_i(0, loop_n, 1):
                    emit_main()
                emit_tail()
        else:
            emit_stage()
            emit_main()
            emit_tail()

    nc.compile()
    return nc


def _get(reps: int = 1, loop_n: int = 1, **kw):
    key = (reps, loop_n, tuple(sorted(kw.items())))
    if key not in _cached:
        _cached[key] = _build(reps, loop_n, **kw)
    return _cached[key]


def kernel(input1: np.ndarray, input2: np.ndarray, _trace: bool = False, **kw):
    nc = _get(**kw)
    input1 = np.ascontiguousarray(np.asarray(input1, dtype=np.float32))
    input2 = np.ascontiguousarray(np.asarray(input2, dtype=np.float32))
    in_maps = [{"x1": input1[b], "x2": input2[b]} for b in range(B)]
    res = run_bass_kernel_spmd(nc, in_maps, core_ids=list(range(B)), trace=_trace)
    losses = []
    for b in range(B):
        r = res.results[b]
        losses.append(
            r["outc"].mean(dtype=np.float64) + r["outr"].mean(dtype=np.float64)
        )
    out = np.float32(np.mean(losses))
    if _trace:
        return out, res
    return out
